# revision 1
# baseline (speedup 1.0000x reference)
"""Trainium2 Bass kernel for BinaryGroupConv block (8-core SPMD, batch-sharded).

For x:(32,256,56,56), w1:(256,64,3,3), w2:(256,256,1,1):
    out = bn1(conv2d(sign(x), sign(w1), s2 p1 g4)) + maxpool3x3s2p1(x)
    x1  = out
    out = bn2(conv2d(sign(out), sign(w2), 1x1)) + x1
with training-mode (batch-stat) BatchNorm -> sync-BN all-reduce across cores.

Strategy per core (4 images):
  - channels on partitions (2 tiles of 128), pixels on free dim
  - binary convs as bf16 matmuls (x-side sign = +/-1 via ACT Sign, w-side
    +/-0.5 via DVE is_ge; products exact, fp32 PSUM accumulation exact)
  - grouped 3x3 conv: 9 taps x block-diagonal [128,128] weights, PSUM accum
  - maxpool separable on DVE in exact fp32
  - per-channel stats via bn_stats/bn_aggr; 2KB AllReduce for sync-BN
"""

import contextlib
import sys

import numpy as np

sys.path.insert(0, "/opt/trn_rl_repo")

import concourse.bass as bass
import concourse.tile as tile
from concourse import bacc, mybir
from concourse.bass import ts
from concourse.bass_utils import run_bass_kernel_spmd
from concourse.masks import make_identity

F32 = mybir.dt.float32
BF16 = mybir.dt.bfloat16
AF = mybir.ActivationFunctionType
OP = mybir.AluOpType

EPS = 1e-5
C = 256
H = 56
HO = 28
PIX = HO * HO  # 784
NCHUNK = 392  # matmul/psum N-tile (14 output rows)
RPC = 14  # output rows per chunk
SC1 = 2.0  # conv1: x-sign +/-1 (ACT), w-sign +/-0.5 -> y_true = 2*y_q
SC2 = 4.0  # conv2: z-sign +/-0.5 (DVE), w-sign +/-0.5 -> y_true = 4*y_q


def build_nc(n_loc: int, n_cores: int):
    nc = bacc.Bacc(
        "TRN2",
        target_bir_lowering=False,
        debug=False,
        enable_asserts=False,
        num_devices=n_cores,
    )
    x_d = nc.dram_tensor("x", [n_loc, C, H, H], F32, kind="ExternalInput").ap()
    w1_d = nc.dram_tensor("w1", [C, 64, 3, 3], F32, kind="ExternalInput").ap()
    w2_d = nc.dram_tensor("w2", [C, C, 1, 1], F32, kind="ExternalInput").ap()
    g1_d = nc.dram_tensor("gamma1", [C], F32, kind="ExternalInput").ap()
    b1_d = nc.dram_tensor("beta1", [C], F32, kind="ExternalInput").ap()
    g2_d = nc.dram_tensor("gamma2", [C], F32, kind="ExternalInput").ap()
    b2_d = nc.dram_tensor("beta2", [C], F32, kind="ExternalInput").ap()
    out_d = nc.dram_tensor("out", [n_loc, C, HO, HO], F32, kind="ExternalOutput").ap()

    with tile.TileContext(nc) as tc:
        kernel_body(
            tc, out_d, x_d, w1_d, w2_d, (g1_d, b1_d, g2_d, b2_d), n_loc, n_cores
        )

    nc.compile()
    return nc


def kernel_body(tc, out_d, x_d, w1_d, w2_d, gb_d, n_loc, n_cores):
    nc = tc.nc
    g1_d, b1_d, g2_d, b2_d = gb_d
    n_units = n_loc * 2
    npix_loc = n_loc * PIX
    npix_glob = npix_loc * n_cores

    ctx = contextlib.ExitStack()
    with ctx:
        singles = ctx.enter_context(tc.tile_pool(name="singles", bufs=1))
        xf_pool = ctx.enter_context(tc.tile_pool(name="xf", bufs=3))
        xs_pool = ctx.enter_context(tc.tile_pool(name="xs", bufs=2))
        rm_pool = ctx.enter_context(tc.tile_pool(name="rmax", bufs=n_units))
        mp_pool = ctx.enter_context(tc.tile_pool(name="mp", bufs=n_units))
        y1_pool = ctx.enter_context(tc.tile_pool(name="y1", bufs=n_units))
        zs_pool = ctx.enter_context(tc.tile_pool(name="zs", bufs=n_units))
        tiny = ctx.enter_context(tc.tile_pool(name="tiny", bufs=16))
        dram = ctx.enter_context(tc.tile_pool(name="dram", bufs=4, space="DRAM"))

        # Dummy tiny AllReduce issued first: absorbs the cross-core launch
        # skew + collectives-firmware warmup concurrently with phase-1 compute,
        # so the real sync-BN all-reduces are fast.
        warm = tiny.tile([128, 1], F32, tag="warm", name="warm")
        nc.gpsimd.memset(warm, 0.0)
        cc_warm_in = dram.tile([128, 1], F32, tag="cc_warm_in", name="cc_warm_in")
        cc_warm_out = dram.tile([128, 1], F32, tag="cc_warm_out", name="cc_warm_out")
        nc.sync.dma_start(out=cc_warm_in, in_=warm)
        nc.gpsimd.collective_compute(
            "AllReduce",
            OP.add,
            replica_groups=[list(range(n_cores))],
            ins=[cc_warm_in.opt()],
            outs=[cc_warm_out.opt()],
        )

        # prefetch the first input tiles so the big DMA stream starts cold-ramp
        # immediately; weight prep overlaps it
        xf_pre = []
        n_pre = min(3, n_units)
        for u in range(n_pre):
            n, t = u // 2, u % 2
            xfp = xf_pool.tile([128, H, H], F32, tag="xf", name=f"xfp_{u}")
            nc.sync.dma_start(out=xfp, in_=x_d[n, ts(t, 128)])
            xf_pre.append(xfp)

        # ---------------- weight prep ----------------
        # w1 grouped-conv weights -> 9 block-diagonal [128,128] lhsT per part-tile
        lhsT1 = [singles.tile([128, 9, 128], BF16, tag=f"lhsT1_{t}", name=f"lhsT1_{t}") for t in range(2)]
        with tc.tile_pool(name="wprep", bufs=1) as wprep, tc.tile_pool(
            name="tr_psum", bufs=2, space="PSUM"
        ) as tr_psum:
            ident = singles.tile([128, 128], BF16)
            make_identity(nc, ident)
            for t in range(2):
                # natural [cout, (cin tap)] load: contiguous, fast
                w1nat = wprep.tile([128, 64, 9], F32, tag="w1nat", name="w1nat")
                nc.sync.dma_start(
                    out=w1nat,
                    in_=w1_d[ts(t, 128)].rearrange("co ci kh kw -> co ci (kh kw)"),
                )
                w1ns = wprep.tile([128, 64, 9], BF16, tag="w1ns", name="w1ns")
                nc.vector.tensor_scalar(
                    out=w1ns, in0=w1nat, scalar1=0.0, scalar2=0.5,
                    op0=OP.is_ge, op1=OP.subtract,
                )
                nc.gpsimd.memset(lhsT1[t], 0.0)
                for tap in range(9):
                    trf = tr_psum.tile([128, 128], BF16, tag="trw", name=f"trw_{t}_{tap}")
                    nc.tensor.transpose(trf[0:64], w1ns[:, :, tap], ident)
                    nc.tensor.transpose(trf[64:128], w1ns[:, :, tap], ident)
                    nc.scalar.copy(
                        out=lhsT1[t][0:64, tap, 0:64], in_=trf[0:64, 0:64]
                    )
                    nc.vector.tensor_copy(
                        out=lhsT1[t][64:128, tap, 64:128], in_=trf[64:128, 64:128]
                    )

            # w2 1x1 weights: natural load, binarize, PE-transpose into lhsT
            w2lhsT = [singles.tile([128, 256], BF16, tag=f"w2lhsT_{k}", name=f"w2lhsT_{k}") for k in range(2)]
            for mt in range(2):
                w2nat = wprep.tile([128, 256], F32, tag="w2nat", name="w2nat")
                nc.sync.dma_start(out=w2nat, in_=w2_d[ts(mt, 128), :, 0, 0])
                w2s = wprep.tile([128, 256], BF16, tag="w2s", name="w2s")
                nc.vector.tensor_scalar(
                    out=w2s, in0=w2nat, scalar1=0.0, scalar2=0.5,
                    op0=OP.is_ge, op1=OP.subtract,
                )
                for kt in range(2):
                    tr = tr_psum.tile([128, 128], BF16, tag="trw", name=f"tr_{mt}_{kt}")
                    nc.tensor.transpose(tr, w2s[:, ts(kt, 128)], ident)
                    nc.scalar.copy(out=w2lhsT[kt][:, ts(mt, 128)], in_=tr)

        # gamma/beta/eps per part-tile
        def load_vec(d_ap, name):
            tiles = []
            for t in range(2):
                tl = singles.tile([128, 1], F32, tag=f"vec_{name}_{t}", name=f"vec_{name}_{t}")
                src = bass.AP(
                    tensor=d_ap.tensor,
                    offset=d_ap.offset + 128 * t,
                    ap=[[1, 128], [0, 1]],
                )
                nc.sync.dma_start(out=tl, in_=src)
                tiles.append(tl)
            return tiles

        # per-(img,chunk) bn_stats records, one buffer per part-tile
        bnst1 = [
            singles.tile([128, n_units, 6], F32, tag=f"bnst1_{t}", name=f"bnst1_{t}") for t in range(2)
        ]
        bnst2 = [
            singles.tile([128, n_units, 6], F32, tag=f"bnst2_{t}", name=f"bnst2_{t}") for t in range(2)
        ]

        from concourse.tile import add_dep_helper

        taps = [(kh, kw) for kh in range(3) for kw in range(3)]

        # ------- phase 1: load, binarize, maxpool, conv1, evict, stats -------
        y1_tiles = {}
        mp_tiles = {}
        rm_tiles = {}
        ps_tiles = {}
        with tc.tile_pool(name="psum1", bufs=4, space="PSUM") as psum1:
            for n in range(n_loc):
                for t in range(2):
                    u = 2 * n + t
                    if u < n_pre:
                        xf = xf_pre[u]
                    else:
                        xf = xf_pool.tile([128, H, H], F32, tag="xf", name=f"xf_{u}")
                        nc.sync.dma_start(out=xf, in_=x_d[n, ts(t, 128)])

                    # binarized input, zero-padded with one row/col at top/left
                    xs = xs_pool.tile([128, H + 1, H + 1], BF16)
                    nc.gpsimd.memset(xs[:, 0, :], 0.0)
                    nc.gpsimd.memset(xs[:, 1:, 0], 0.0)
                    nc.scalar.sign(out=xs[:, 1:, 1:], in_=xf)

                    # maxpool 3x3 s2 p1, exact fp32, separable (rows then cols)
                    rmax = rm_pool.tile([128, HO, H], F32)
                    nc.vector.tensor_tensor(
                        out=rmax, in0=xf[:, 0:H:2], in1=xf[:, 1:H:2], op=OP.max
                    )
                    nc.vector.tensor_tensor(
                        out=rmax[:, 1:], in0=rmax[:, 1:],
                        in1=xf[:, 1 : H - 2 : 2], op=OP.max,
                    )
                    rm_tiles[(n, t)] = rmax

                    # conv1: 9 taps, block-diag [128,128] bf16, PSUM accumulation
                    ps = [
                        psum1.tile([128, RPC, HO], F32, tag=f"ps1_{c}", name=f"ps1_{n}_{t}_{c}")
                        for c in range(2)
                    ]
                    for tap_i, (kh, kw) in enumerate(taps):
                        for c in range(2):
                            rhs = xs[
                                :,
                                28 * c + kh : 28 * c + kh + 27 : 2,
                                kw : kw + 55 : 2,
                            ]
                            nc.tensor.matmul(
                                ps[c],
                                lhsT1[t][:, tap_i, :],
                                rhs,
                                start=(tap_i == 0),
                                stop=(tap_i == 8),
                            )
                    ps_tiles[(n, t)] = ps
            # trailing evict+stats loop: keeps ACT/DVE queues free of
            # weight-blocked work during the load/pool ramp
            for n in range(n_loc):
                for t in range(2):
                    ps = ps_tiles[(n, t)]
                    y1 = y1_pool.tile([128, PIX], F32)
                    for c in range(2):
                        nc.scalar.copy(
                            out=y1[:, ts(c, NCHUNK)].rearrange(
                                "p (a b) -> p a b", a=RPC
                            ),
                            in_=ps[c],
                        )
                        nc.vector.bn_stats(
                            out=bnst1[t][:, n * 2 + c, :], in_=y1[:, ts(c, NCHUNK)]
                        )
                    y1_tiles[(n, t)] = y1

        # ---- local aggregate -> (S, SS) -> AllReduce ----
        def stats_allreduce(bnst, tag):
            allin = tiny.tile([128, 4], F32, tag=f"allin_{tag}", name=f"allin_{tag}")
            for t in range(2):
                mv = tiny.tile([128, 2], F32, tag=f"mv_{tag}_{t}", name=f"mv_{tag}_{t}")
                nc.vector.bn_aggr(out=mv, in_=bnst[t])
                m2 = tiny.tile([128, 1], F32, tag=f"m2_{tag}_{t}", name=f"m2_{tag}_{t}")
                nc.vector.tensor_tensor(
                    out=m2, in0=mv[:, 0:1], in1=mv[:, 0:1], op=OP.mult
                )
                vp = tiny.tile([128, 1], F32, tag=f"vp_{tag}_{t}", name=f"vp_{tag}_{t}")
                nc.vector.tensor_tensor(out=vp, in0=mv[:, 1:2], in1=m2, op=OP.add)
                nc.vector.tensor_scalar_mul(
                    out=allin[:, 2 * t : 2 * t + 1], in0=mv[:, 0:1],
                    scalar1=float(npix_loc),
                )
                nc.vector.tensor_scalar_mul(
                    out=allin[:, 2 * t + 1 : 2 * t + 2], in0=vp,
                    scalar1=float(npix_loc),
                )
            cc_in = dram.tile([128, 4], F32, tag=f"ccin_{tag}", name=f"ccin_{tag}")
            cc_out = dram.tile([128, 4], F32, tag=f"ccout_{tag}", name=f"ccout_{tag}")
            gate = nc.vector.tensor_copy(out=allin[:, 0:1], in_=allin[:, 0:1])
            nc.sync.dma_start(out=cc_in, in_=allin)
            nc.gpsimd.collective_compute(
                "AllReduce",
                OP.add,
                replica_groups=[list(range(n_cores))],
                ins=[cc_in.opt()],
                outs=[cc_out.opt()],
            )
            gst = tiny.tile([128, 4], F32, tag=f"gst_{tag}", name=f"gst_{tag}")
            nc.sync.dma_start(out=gst, in_=cc_out)
            return gst, gate

        def bn_coeffs(gst, gam, bet, tag, SC):
            """Global (S,SS) -> per part-tile (a_eff, b_eff): out = y_q*a_eff + b_eff."""
            coeffs = []
            for t in range(2):
                meanq = tiny.tile([128, 1], F32, tag=f"mq_{tag}_{t}", name=f"mq_{tag}_{t}")
                nc.vector.tensor_scalar_mul(
                    out=meanq, in0=gst[:, 2 * t : 2 * t + 1], scalar1=1.0 / npix_glob
                )
                ssq = tiny.tile([128, 1], F32, tag=f"ssq_{tag}_{t}", name=f"ssq_{tag}_{t}")
                nc.vector.tensor_scalar_mul(
                    out=ssq, in0=gst[:, 2 * t + 1 : 2 * t + 2], scalar1=1.0 / npix_glob
                )
                m2 = tiny.tile([128, 1], F32, tag=f"cm2_{tag}_{t}", name=f"cm2_{tag}_{t}")
                nc.vector.tensor_tensor(out=m2, in0=meanq, in1=meanq, op=OP.mult)
                varq = tiny.tile([128, 1], F32, tag=f"varq_{tag}_{t}", name=f"varq_{tag}_{t}")
                nc.vector.tensor_tensor(out=varq, in0=ssq, in1=m2, op=OP.subtract)
                vart = tiny.tile([128, 1], F32, tag=f"vart_{tag}_{t}", name=f"vart_{tag}_{t}")
                nc.vector.tensor_scalar_mul(out=vart, in0=varq, scalar1=SC * SC)
                sd = tiny.tile([128, 1], F32, tag=f"sd_{tag}_{t}", name=f"sd_{tag}_{t}")
                nc.scalar.activation(out=sd, in_=vart, func=AF.Sqrt, bias=eps_t)
                r = tiny.tile([128, 1], F32, tag=f"r_{tag}_{t}", name=f"r_{tag}_{t}")
                nc.vector.reciprocal(out=r, in_=sd)
                rg = tiny.tile([128, 1], F32, tag=f"rg_{tag}_{t}", name=f"rg_{tag}_{t}")
                nc.vector.tensor_tensor(out=rg, in0=r, in1=gam[t], op=OP.mult)
                a_eff = tiny.tile([128, 1], F32, tag=f"aeff_{tag}_{t}", name=f"aeff_{tag}_{t}")
                nc.vector.tensor_scalar_mul(out=a_eff, in0=rg, scalar1=SC)
                mrg = tiny.tile([128, 1], F32, tag=f"mrg_{tag}_{t}", name=f"mrg_{tag}_{t}")
                nc.vector.tensor_tensor(out=mrg, in0=meanq, in1=rg, op=OP.mult)
                nmrg = tiny.tile([128, 1], F32, tag=f"nmrg_{tag}_{t}", name=f"nmrg_{tag}_{t}")
                nc.vector.tensor_scalar_mul(out=nmrg, in0=mrg, scalar1=-SC)
                b_eff = tiny.tile([128, 1], F32, tag=f"beff_{tag}_{t}", name=f"beff_{tag}_{t}")
                nc.vector.tensor_tensor(out=b_eff, in0=nmrg, in1=bet[t], op=OP.add)
                coeffs.append((a_eff, b_eff))
            return coeffs

        # per-channel affine params: tiny DMAs, deferred so they never
        # delay the big input streams on the cold DMA path
        g1_t = load_vec(g1_d, "g1")
        b1_t = load_vec(b1_d, "b1")
        g2_t = load_vec(g2_d, "g2")
        b2_t = load_vec(b2_d, "b2")
        eps_t = singles.tile([128, 1], F32)
        nc.vector.memset(eps_t, EPS)

        gst1, s1_gate = stats_allreduce(bnst1, "s1")

        # deferred maxpool column stage: fills the all-reduce bubble
        for n in range(n_loc):
            for t in range(2):
                if (n, t) not in rm_tiles:
                    continue
                rmax = rm_tiles[(n, t)]
                mp = mp_pool.tile([128, HO, HO], F32, tag='mpy2', name=f'mp_{n}_{t}')
                i1 = nc.vector.tensor_tensor(
                    out=mp, in0=rmax[:, :, 0:H:2], in1=rmax[:, :, 1:H:2], op=OP.max
                )
                add_dep_helper(i1.ins, s1_gate.ins, sync=False,
                               reason="fill AR1 bubble: pool stage-2 after stats")
                nc.vector.tensor_tensor(
                    out=mp[:, :, 1:], in0=mp[:, :, 1:],
                    in1=rmax[:, :, 1 : H - 2 : 2], op=OP.max,
                )
                mp_tiles[(n, t)] = mp

        c1 = bn_coeffs(gst1, g1_t, b1_t, "s1", SC1)

        # ------- phase 2: bn1 apply + maxpool add, binarize -> zs -------
        zs_tiles = {}
        for n in range(n_loc):
            for t in range(2):
                y1 = y1_tiles[(n, t)]
                a_eff, b_eff = c1[t]
                nc.scalar.activation(
                    out=y1, in_=y1, func=AF.Identity, bias=b_eff, scale=a_eff
                )
                nc.vector.tensor_tensor(
                    out=y1, in0=y1,
                    in1=mp_tiles[(n, t)].rearrange("p a b -> p (a b)"), op=OP.add,
                )
                # y1 now holds x1
                zsb = zs_pool.tile([128, PIX], BF16)
                nc.vector.tensor_scalar(
                    out=zsb, in0=y1, scalar1=0.0, scalar2=0.5,
                    op0=OP.is_ge, op1=OP.subtract,
                )
                zs_tiles[(n, t)] = zsb

        # ------- phase 3: conv2 (1x1), evict, stats -------
        y2_tiles = {}
        with tc.tile_pool(name="psum2", bufs=4, space="PSUM") as psum2:
            for n in range(n_loc):
                for mt in range(2):
                    ps = [
                        psum2.tile([128, NCHUNK], F32, tag=f"ps2_{c}", name=f"ps2_{n}_{mt}_{c}") for c in range(2)
                    ]
                    for kt in range(2):
                        for c in range(2):
                            nc.tensor.matmul(
                                ps[c],
                                w2lhsT[kt][:, ts(mt, 128)],
                                zs_tiles[(n, kt)][:, ts(c, NCHUNK)],
                                start=(kt == 0),
                                stop=(kt == 1),
                            )
                    y2 = mp_pool.tile([128, PIX], F32, tag='mpy2', name=f'y2_{n}_{mt}')
                    for c in range(2):
                        nc.scalar.copy(out=y2[:, ts(c, NCHUNK)], in_=ps[c])
                        nc.vector.bn_stats(
                            out=bnst2[mt][:, n * 2 + c, :], in_=y2[:, ts(c, NCHUNK)]
                        )
                    y2_tiles[(n, mt)] = y2

        gst2, _ = stats_allreduce(bnst2, "s2")
        c2 = bn_coeffs(gst2, g2_t, b2_t, "s2", SC2)

        # ------- phase 4: bn2 apply + residual + store -------
        for n in range(n_loc):
            for mt in range(2):
                y2 = y2_tiles[(n, mt)]
                a_eff, b_eff = c2[mt]
                if (2 * n + mt) % 2 == 0:
                    nc.scalar.activation(
                        out=y2, in_=y2, func=AF.Identity, bias=b_eff, scale=a_eff
                    )
                else:
                    nc.vector.tensor_scalar(
                        out=y2, in0=y2, scalar1=a_eff, scalar2=b_eff,
                        op0=OP.mult, op1=OP.add,
                    )
                nc.vector.tensor_tensor(
                    out=y2, in0=y2, in1=y1_tiles[(n, mt)], op=OP.add
                )
                nc.gpsimd.dma_start(
                    out=out_d[n, ts(mt, 128)],
                    in_=y2.rearrange("p (h w) -> p h w", h=HO),
                )


_NC_CACHE = {}


def get_nc(n_loc=4, n_cores=8):
    key = (n_loc, n_cores)
    if key not in _NC_CACHE:
        _NC_CACHE[key] = build_nc(n_loc, n_cores)
    return _NC_CACHE[key]


def kernel(**inputs):
    n_cores = 8
    x = np.asarray(inputs["x"], dtype=np.float32)
    n_loc = x.shape[0] // n_cores
    nc = get_nc(n_loc, n_cores)
    shared = {
        k: np.asarray(inputs[k], dtype=np.float32)
        for k in ("w1", "w2", "gamma1", "beta1", "gamma2", "beta2")
    }
    in_maps = [{"x": x[i * n_loc : (i + 1) * n_loc], **shared} for i in range(n_cores)]
    res = run_bass_kernel_spmd(nc, in_maps, core_ids=list(range(n_cores)))
    return np.concatenate([res.results[i]["out"] for i in range(n_cores)], axis=0)

